# revision 1
# baseline (speedup 1.0000x reference)
"""DeformConv2d TRN2 kernel: build + host prep + SPMD runner.

Layout/algorithm summary (per core; 8 cores = 4 batches x 2 row-halves):
  - offset conv (3x3, 18 out ch) as 18 K-tile matmuls from shifted APs of
    padded c-major x in SBUF -> offsets [18, 2048] in PSUM -> +bias -> SBUF.
  - PE-transpose offsets to [n-part, 18], compute bilinear ints/fracs/weights
    on DVE in [128, 16, 18] layout (per-partition = per-sample).
  - int16 flat indices -> DRAM -> reload in dma_gather's wrapped [16, S]
    layout (replicated over 8 Q7 cores).
  - dma_gather (transpose=False) fetches 2-adjacent-pixel rows (512 fp16) of
    the [5185, 256] fp16 padded pixel-major table: G^T tiles [n'128, 512].
  - DVE tensor_scalar per-partition scaling by bilinear weights, then PE
    matmul vs fp16 identity = scaled transpose, PSUM-accumulating the 4
    bilinear terms -> sampled [(kk,ch) x 128c, n] fp16.
  - main GEMM: 18 K-tiles x 2 o-halves, +bias, DMA out [256, 2048] f32.

Zero-padding of the table by 4 rows/cols emulates the reference's
valid-masking exactly for |excursion| <= 4; p is clamped to [0, 70.999] in
padded coords so larger offsets also read only zero-pad rows (-> exact 0).
"""

import sys

sys.path.insert(0, "/opt/trn_rl_repo")

import numpy as np

import bass_rust
import concourse.bass as bass
import concourse.bacc as bacc
import concourse.mybir as mybir
import concourse.tile as tile
from concourse import bass_utils
from concourse.tile_rust import add_dep_helper

P = 128
KK = 9
C = 256
H = W = 64
HO = 32          # rows per core (half image)
NS = HO * W      # samples per core = 2048
NT = NS // P     # 16 subtiles of 128 samples
PAD = 4
WP = 72          # padded width/height
TBL = WP * WP    # 5184 pixel rows (+1 safety row)
F16 = mybir.dt.float16
F32 = mybir.dt.float32
I16 = mybir.dt.int16


def build(debug_outputs=False):
    nc = bacc.Bacc("TRN2", num_devices=8, debug=False)

    xpad = nc.dram_tensor("xpad", [TBL + 1, C], F16, kind="ExternalInput")
    xchw = nc.dram_tensor("xchw", [2, P, 34 * WP], F16, kind="ExternalInput")
    wre = nc.dram_tensor("wre", [18, P, C], F16, kind="ExternalInput")
    owre = nc.dram_tensor("owre", [18, P, 18], F16, kind="ExternalInput")
    basep4 = nc.dram_tensor("basep4", [P, NT * 18], F32, kind="ExternalInput")
    idn16d = nc.dram_tensor("idn16", [P, P], F16, kind="ExternalInput")
    idn32d = nc.dram_tensor("idn32", [P, P], F32, kind="ExternalInput")
    obcold = nc.dram_tensor("obcol", [P, 1], F32, kind="ExternalInput")
    bcolsd = nc.dram_tensor("bcols", [P, 2], F32, kind="ExternalInput")

    idxscr = nc.dram_tensor("idxscr", [36, 1024], I16, kind="Internal")
    out = nc.dram_tensor("out", [C, NS], F32, kind="ExternalOutput")
    if debug_outputs:
        dbg_off = nc.dram_tensor("dbg_off", [18, NS], F32, kind="ExternalOutput")
        dbg_w4 = nc.dram_tensor("dbg_w4", [P, NT * KK * 4], F32, kind="ExternalOutput")
        dbg_idx = nc.dram_tensor("dbg_idx", [P, NT * KK * 2], F32, kind="ExternalOutput")
        dbg_smp = nc.dram_tensor("dbg_smp", [P, 18 * NS], F16, kind="ExternalOutput")

    from contextlib import ExitStack

    with tile.TileContext(nc) as tc, ExitStack() as es:
        cst = es.enter_context(tc.tile_pool(name="cst", bufs=1))
        sb = es.enter_context(tc.tile_pool(name="sb", bufs=1))
        smpp = es.enter_context(tc.tile_pool(name="smp", bufs=2))
        gpool = es.enter_context(tc.tile_pool(name="gp", bufs=3))
        sclp = es.enter_context(tc.tile_pool(name="scl", bufs=6))
        otp = es.enter_context(tc.tile_pool(name="ot", bufs=4))
        psAB = ExitStack()
        psA = psAB.enter_context(tc.tile_pool(name="psA", bufs=2, space="PSUM"))

        # ---- load constants
        t_xchw = cst.tile([P, 2, 34 * WP], F16)
        nc.sync.dma_start(out=t_xchw[:], in_=xchw.ap().transpose([1, 0, 2]))
        t_wre = cst.tile([P, 18, C], F16)
        nc.sync.dma_start(out=t_wre[:], in_=wre.ap().transpose([1, 0, 2]))
        t_owre = cst.tile([P, 18, 18], F16)
        nc.sync.dma_start(out=t_owre[:], in_=owre.ap().transpose([1, 0, 2]))
        t_base = cst.tile([P, NT * 18], F32)
        nc.sync.dma_start(out=t_base[:], in_=basep4.ap())
        t_idn16 = cst.tile([P, P], F16)
        nc.sync.dma_start(out=t_idn16[:], in_=idn16d.ap())
        t_idn32 = cst.tile([P, P], F32)
        nc.sync.dma_start(out=t_idn32[:], in_=idn32d.ap())
        t_obcol = cst.tile([P, 1], F32)
        nc.sync.dma_start(out=t_obcol[:], in_=obcold.ap())
        t_bcols = cst.tile([P, 2], F32)
        nc.sync.dma_start(out=t_bcols[:], in_=bcolsd.ap())

        # ---- stage A: offset conv -> off_sb [18, 2048] f32
        off_sb = sb.tile([P, NS], F32, tag="offsb")
        for blk in range(4):
            ps = psA.tile([P, 512], F32, tag="psoff")
            for t in range(18):
                kk, ch = t // 2, t % 2
                ky, kx = kk // 3, kk % 3
                rhs = t_xchw[:, ch, :].rearrange("p (r w) -> p r w", w=WP)[
                    :, blk * 8 + ky : blk * 8 + ky + 8, kx + 3 : kx + 3 + W
                ]
                nc.tensor.matmul(
                    ps[0:18, :],
                    lhsT=t_owre[:, t, :],
                    rhs=rhs,
                    start=(t == 0),
                    stop=(t == 17),
                )
            nc.scalar.add(off_sb[0:18, blk * 512 : (blk + 1) * 512], ps[0:18, :], t_obcol[0:18, :])
        if debug_outputs:
            nc.sync.dma_start(out=dbg_off.ap(), in_=off_sb[0:18, :])

        # ---- stage B: transpose to offT [128, 16, 18] f32
        offT = sb.tile([P, NT, 18], F32, tag="offT")
        for st in range(NT):
            pst = psA.tile([P, 18], F32, tag="pstr")
            nc.tensor.transpose(
                pst[:, 0:18],
                in_=off_sb[0:18, st * P : (st + 1) * P],
                identity=t_idn32[0:18, 0:18],
            )
            nc.vector.tensor_copy(offT[:, st, :], pst[:, 0:18])

        # ---- stage C: bilinear math (all [128, 16*18] f32)
        AL = mybir.AluOpType
        pP4 = sb.tile([P, NT, 18], F32, tag="pP4")
        nc.vector.tensor_add(pP4[:], offT[:], t_base[:].rearrange("p (s d) -> p s d", d=18))
        pc = sb.tile([P, NT, 18], F32, tag="pc")
        nc.vector.tensor_scalar(pc[:], pP4[:], 0.0, 70.999, op0=AL.max, op1=AL.min)
        # floor via round-to-nearest int cast of (pc - 0.5): HW f32->i32 is RNE.
        # Exact-integer pc can floor to pc-1 with frac==1.0 - equivalent bilinear.
        i32 = sb.tile([P, NT, 18], mybir.dt.int32, tag="i32")
        nc.vector.tensor_scalar_add(i32[:], pc[:], -0.5)
        ipart = sb.tile([P, NT, 18], F32, tag="ipart")
        nc.vector.tensor_copy(ipart[:], i32[:])
        frac = sb.tile([P, NT, 18], F32, tag="frac")
        nc.vector.tensor_sub(frac[:], pc[:], ipart[:])
        omf = sb.tile([P, NT, 18], F32, tag="omf")
        nc.vector.tensor_scalar(omf[:], frac[:], -1.0, 1.0, op0=AL.mult, op1=AL.add)

        def ysl(t):  # [128, 16, 9] strided views (d = 2kk + {0:y, 1:x})
            return t[:].rearrange("p s (k two) -> p s k two", two=2)[:, :, :, 0]

        def xsl(t):
            return t[:].rearrange("p s (k two) -> p s k two", two=2)[:, :, :, 1]

        # w4 [128, 16, 9, 4]; term q = j*2+pix: (wy_j) * (wx_pix)
        w4 = sb.tile([P, NT, KK, 4], F32, tag="w4")
        nc.vector.tensor_mul(w4[:, :, :, 0], ysl(omf), xsl(omf))
        nc.vector.tensor_mul(w4[:, :, :, 1], ysl(omf), xsl(frac))
        nc.vector.tensor_mul(w4[:, :, :, 2], ysl(frac), xsl(omf))
        nc.vector.tensor_mul(w4[:, :, :, 3], ysl(frac), xsl(frac))
        if debug_outputs:
            nc.sync.dma_start(out=dbg_w4.ap(), in_=w4[:].rearrange("p a b c -> p (a b c)"))

        # idxf [128, 16, 9, 2]: flat = 72*(y0+4) + (x0+4); j1 = +72
        idxf = sb.tile([P, NT, KK, 2], F32, tag="idxf")
        tmp72 = sb.tile([P, NT, KK], F32, tag="tmp72")
        nc.vector.tensor_scalar_mul(tmp72[:], ysl(ipart), 72.0)
        nc.vector.tensor_add(idxf[:, :, :, 0], tmp72[:], xsl(ipart))
        nc.vector.tensor_scalar_add(idxf[:, :, :, 1], idxf[:, :, :, 0], 72.0)
        if debug_outputs:
            nc.sync.dma_start(out=dbg_idx.ap(), in_=idxf[:].rearrange("p a b c -> p (a b c)"))
        idx16 = sb.tile([P, NT, KK, 2], I16, tag="idx16")
        nc.vector.tensor_copy(idx16[:], idxf[:])

        psAB.close()  # free stage A/B PSUM banks
        psB = es.enter_context(tc.tile_pool(name="psB", bufs=3, space="PSUM"))
        psC = es.enter_context(tc.tile_pool(name="psC", bufs=2, space="PSUM"))

        # ---- stage D: wrap indices via DRAM roundtrip
        # dst element (p, st=(hh,st8), kk, j) -> idxscr[q(hh,kk,j), st8*128 + p]
        # call index q = (kk*2 + j)*2 + hh. Gather slot i = s8*128 + p within a
        # call maps to wrapped (r, s) = (i%16, i//16) = (p%16, s8*8 + p//16).
        # DRAM scratch laid out r-major [r(16), q(36), s(64)] so the readback
        # is contiguous per partition; write side = 8 DMAs sliced by a = p//16.
        d_outs = []
        for st in range(NT):
            hh, s8 = st // 8, st % 8
            dst = bass.AP(idxscr, hh * 1024 + s8 * P, [[1, P], [2048, 18]])
            d_outs.append(
                nc.sync.dma_start(
                    out=dst, in_=idx16[:, st, :, :].rearrange("p k j -> p (k j)")
                )
            )
        # reload wrapped: idxs_sb[r(8rep x 16), q, s] = idxscr[q, s*16 + r%16]
        idxs_sb = sb.tile([P, 36, 64], I16, tag="idxs")
        srcw = bass.AP(idxscr, 0, [[1, 16], [1024, 36], [16, 64]])
        for rep in range(8):
            d_in = nc.sync.dma_start(
                out=idxs_sb[rep * 16 : (rep + 1) * 16, :, :], in_=srcw
            )
            for d_out in d_outs:
                add_dep_helper(d_in.ins, d_out.ins, reason="idx roundtrip RAW")

        # ---- stage E: gather + combine + main GEMM per half
        xpad_src = bass.AP(xpad, 0, [[C, TBL], [1, 2 * C]])
        for hh in range(2):
            sampled = smpp.tile([P, 18, 1024], F16, tag="sampled")
            for kk in range(KK):
                gd = [gpool.tile([P, 8, 512], F16, tag=f"g{j}", name=f"g{j}_{hh}_{kk}") for j in range(2)]
                for j in range(2):
                    q = (kk * 2 + j) * 2 + hh
                    nc.gpsimd.dma_gather(
                        gd[j][:],
                        xpad_src,
                        idxs_sb[:, q, :],
                        num_idxs=1024,
                        num_idxs_reg=1024,
                        elem_size=2 * C,
                        elem_step=C,
                    )
                for g4 in range(2):
                    pss = [psB.tile([P, 512], F32, tag=f"pss{ch}", name=f"pss{ch}_{hh}_{kk}_{g4}") for ch in range(2)]
                    for i4 in range(4):
                        st8 = g4 * 4 + i4
                        st = hh * 8 + st8
                        scl = sclp.tile([P, 2, 512], F16, tag="scl")
                        for j in range(2):
                            for pix in range(2):
                                qq = j * 2 + pix
                                nc.vector.tensor_scalar_mul(
                                    scl[:, j, pix * C : (pix + 1) * C],
                                    gd[j][:, st8, pix * C : (pix + 1) * C],
                                    w4[:, st, kk, qq : qq + 1],
                                )
                        for ch in range(2):
                            for term in range(4):
                                j, pix = term // 2, term % 2
                                nc.tensor.matmul(
                                    pss[ch][:, i4 * P : (i4 + 1) * P],
                                    lhsT=scl[:, j, pix * C + ch * P : pix * C + (ch + 1) * P],
                                    rhs=t_idn16[:],
                                    start=(term == 0),
                                    stop=(term == 3),
                                )
                    for ch in range(2):
                        nc.scalar.copy(
                            sampled[:, kk * 2 + ch, g4 * 512 : (g4 + 1) * 512],
                            pss[ch][:],
                        )
            if debug_outputs:
                nc.sync.dma_start(
                    out=dbg_smp.ap().rearrange("p (t hh n) -> p t hh n", hh=2, n=1024)[:, :, hh, :],
                    in_=sampled[:],
                )
            # main GEMM for this half
            for oh in range(2):
                for blk in range(2):
                    pso = psC.tile([P, 512], F32, tag="pso")
                    for t in range(18):
                        nc.tensor.matmul(
                            pso[:],
                            lhsT=t_wre[:, t, oh * P : (oh + 1) * P],
                            rhs=sampled[:, t, blk * 512 : (blk + 1) * 512],
                            start=(t == 0),
                            stop=(t == 17),
                        )
                    ot = otp.tile([P, 512], F32, tag="ot")
                    nc.scalar.add(ot[:], pso[:], t_bcols[:, oh : oh + 1])
                    nc.sync.dma_start(
                        out=bass.AP(
                            out, oh * P * NS + hh * 1024 + blk * 512, [[NS, P], [1, 512]]
                        ),
                        in_=ot[:],
                    )

    nc.compile()
    return nc


def host_prep(x, weight, bias, offset_w, offset_b):
    """Returns (in_maps list of 8 dicts, assemble fn)."""
    B = x.shape[0]
    xp = np.zeros((B, WP, WP, C), np.float16)
    xp[:, PAD : PAD + H, PAD : PAD + W, :] = x.transpose(0, 2, 3, 1)
    xpad_b = [
        np.concatenate([xp[b].reshape(TBL, C), np.zeros((1, C), np.float16)], 0)
        for b in range(B)
    ]
    # c-major padded image for the offset conv, per (b, hh): rows 32h+3 .. +37
    xcp = xp.transpose(0, 3, 1, 2).reshape(B, 2, P, WP, WP)  # [b, grp, 128, 72, 72]
    wre = np.ascontiguousarray(
        weight.reshape(C, 2, P, 3, 3).transpose(3, 4, 1, 2, 0).reshape(KK * 2, P, C)
    ).astype(np.float16)
    # t = kk*2 + ch ; value = weight[o, ch*128+i, ky, kx]
    owre = np.ascontiguousarray(
        offset_w.reshape(18, 2, P, 3, 3).transpose(3, 4, 1, 2, 0).reshape(KK * 2, P, 18)
    ).astype(np.float16)
    idn16 = np.eye(P, dtype=np.float16)
    idn32 = np.eye(P, dtype=np.float32)
    obcol = np.zeros((P, 1), np.float32)
    obcol[:18, 0] = offset_b
    bcols = np.asarray(bias, np.float32).reshape(2, P).T.copy()  # [128, 2]

    base_all = []
    for hh in range(2):
        base = np.zeros((P, NT, 18), np.float32)
        p = np.arange(P)
        for st in range(NT):
            n = st * P + p
            ho = 32 * hh + n // W
            wo = n % W
            for kk in range(KK):
                ky, kx = kk // 3, kk % 3
                base[:, st, 2 * kk + 0] = ky + ho - 1 + PAD
                base[:, st, 2 * kk + 1] = kx + wo - 1 + PAD
        base_all.append(base.reshape(P, NT * 18))

    in_maps = []
    for core in range(8):
        b, hh = core // 2, core % 2
        in_maps.append(
            {
                "xpad": xpad_b[b],
                "xchw": np.ascontiguousarray(
                    xcp[b, :, :, 32 * hh + 3 : 32 * hh + 37, :].reshape(2, P, 34 * WP)
                ),
                "wre": wre,
                "owre": owre,
                "basep4": base_all[hh],
                "idn16": idn16,
                "idn32": idn32,
                "obcol": obcol,
                "bcols": bcols,
            }
        )

    def assemble(results):
        y = np.empty((B, C, H, W), np.float32)
        for core in range(8):
            b, hh = core // 2, core % 2
            y[b, :, 32 * hh : 32 * (hh + 1), :] = results[core]["out"].reshape(C, HO, W)
        return y

    return in_maps, assemble


_CACHE = {}


def _maybe_reset_devices():
    # Clear any wedged accelerator state left by a previous crashed run.
    try:
        import ctypes
        import jax

        jax.devices()
        lib = ctypes.CDLL("/opt/axon/libaxon_pjrt.so")
        if hasattr(lib, "axon_reset"):
            lib.axon_reset.restype = ctypes.c_int64
            lib.axon_reset()
    except Exception:
        pass


def kernel(x, weight, bias, offset_w, offset_b, trace=False):
    if "nc" not in _CACHE:
        _maybe_reset_devices()
        _CACHE["nc"] = build()
    nc = _CACHE["nc"]
    in_maps, assemble = host_prep(
        np.asarray(x), np.asarray(weight), np.asarray(bias),
        np.asarray(offset_w), np.asarray(offset_b),
    )
    res = bass_utils.run_bass_kernel_spmd(
        nc, in_maps, core_ids=list(range(8)), trace=trace
    )
    out = assemble(res.results)
    _CACHE["last_exec_time_ns"] = res.exec_time_ns
    return out



# revision 8
# speedup vs baseline: 1.3730x; 1.3730x over previous
"""DeformConv2d TRN2 kernel: build + host prep + SPMD runner.

Layout/algorithm summary (per core; 8 cores = 4 batches x 2 row-halves):
  - offset conv (3x3, 18 out ch) as 18 K-tile matmuls from shifted APs of
    padded c-major x in SBUF -> offsets [18, 2048] in PSUM -> +bias -> SBUF.
  - PE-transpose offsets to [n-part, 18], compute bilinear ints/fracs/weights
    on DVE in [128, 16, 9|36] layout (per-partition = per-sample).
  - Gather indices are built ON-CHIP in the wrapped [16-partition, s] layout
    dma_gather requires: per half, PE-transpose idx [128, 72] -> [72, 128],
    then 8 transposes whose stationary operand uses a stride-0 column AP to
    replicate 16 sample-columns across all 128 partitions; strided DVE copies
    (f32->i16 cast) assemble idxs16[128, kk, st8, a]. No DRAM roundtrip.
  - dma_gather on the PAIR table xpad2[r] = [pix r | pix r+72] with
    elem_size=1024, elem_step=512: ONE descriptor per (sample, tap) fetches
    all 4 bilinear corners (2KB). 9 gathers per half of num_idxs=1024.
  - combine: 4 fused DVE ops per (st8, kk): acc = g_q * w_q + acc
    (scalar_tensor_tensor) -> sampled row [128, 256] f16; 2 PE transposes
    (vs identity) per (st8, kk) move it to [ch, n]; Act copies PSUM->sampled.
  - main GEMM streams: K-tile t = kk*2+ch accumulated into 4 PSUM banks per
    half as each tap arrives; +bias, DMA out [256, 2048] f32.

Zero-padding of the table by 4 rows/cols emulates the reference's
valid-masking exactly for |excursion| <= 4; p is clamped to [0, 70.999] in
padded coords so larger offsets also read only zero-pad rows (-> exact 0).
"""

import sys

sys.path.insert(0, "/opt/trn_rl_repo")

import numpy as np

import bass_rust
import concourse.bass as bass
import concourse.bacc as bacc
import concourse.mybir as mybir
import concourse.tile as tile
from concourse import bass_utils

P = 128
KK = 9
C = 256
H = W = 64
HO = 32          # rows per core (half image)
NS = HO * W      # samples per core = 2048
NT = NS // P     # 16 subtiles of 128 samples
PAD = 4
WP = 72          # padded width/height
NPIX = WP * WP   # 5184 pixels
TBL2 = 5113      # pair-table rows (idx <= 5110, fetch spans rows idx..idx+1)
F16 = mybir.dt.float16
F32 = mybir.dt.float32
I16 = mybir.dt.int16


def build(debug_outputs=False):
    nc = bacc.Bacc("TRN2", num_devices=8, debug=False)

    xpad2 = nc.dram_tensor("xpad2", [TBL2, 2 * C], F16, kind="ExternalInput")
    xchw = nc.dram_tensor("xchw", [2, P, 34 * WP], F16, kind="ExternalInput")
    wre = nc.dram_tensor("wre", [18, P, C], F16, kind="ExternalInput")
    owre = nc.dram_tensor("owre", [18, P, 18], F16, kind="ExternalInput")
    basep4 = nc.dram_tensor("basep4", [P, NT * 18], F32, kind="ExternalInput")
    idn16d = nc.dram_tensor("idn16", [P, P], F16, kind="ExternalInput")
    idn32d = nc.dram_tensor("idn32", [P, P], F32, kind="ExternalInput")
    obcold = nc.dram_tensor("obcol", [P, 1], F32, kind="ExternalInput")
    bcolsd = nc.dram_tensor("bcols", [P, 2], F32, kind="ExternalInput")

    out = nc.dram_tensor("out", [C, NS], F32, kind="ExternalOutput")
    if debug_outputs:
        dbg_off = nc.dram_tensor("dbg_off", [18, NS], F32, kind="ExternalOutput")
        dbg_w4 = nc.dram_tensor("dbg_w4", [P, NT * KK * 4], F32, kind="ExternalOutput")
        dbg_idx = nc.dram_tensor("dbg_idx", [P, 2 * KK * 64], I16, kind="ExternalOutput")
        dbg_smp = nc.dram_tensor("dbg_smp", [P, 18 * NS], F16, kind="ExternalOutput")

    from contextlib import ExitStack

    AL = mybir.AluOpType

    with tile.TileContext(nc) as tc, ExitStack() as es:
        cst = es.enter_context(tc.tile_pool(name="cst", bufs=1))
        sb = es.enter_context(tc.tile_pool(name="sb", bufs=1))
        smpp = es.enter_context(tc.tile_pool(name="smp", bufs=2))
        gpool = es.enter_context(tc.tile_pool(name="gp", bufs=3))
        sclp = es.enter_context(tc.tile_pool(name="scl", bufs=8))
        otp = es.enter_context(tc.tile_pool(name="ot", bufs=4))
        psAB = ExitStack()
        psA = psAB.enter_context(tc.tile_pool(name="psA", bufs=2, space="PSUM"))
        psT = psAB.enter_context(tc.tile_pool(name="psT", bufs=2, space="PSUM"))

        # ---- load constants
        t_xchw = cst.tile([P, 2, 34 * WP], F16)
        nc.sync.dma_start(out=t_xchw[:], in_=xchw.ap().transpose([1, 0, 2]))
        t_owre = cst.tile([P, 18, 18], F16)
        nc.sync.dma_start(out=t_owre[:], in_=owre.ap().transpose([1, 0, 2]))
        t_base = cst.tile([P, NT * 18], F32)
        nc.sync.dma_start(out=t_base[:], in_=basep4.ap())
        t_idn16 = cst.tile([P, P], F16)
        nc.sync.dma_start(out=t_idn16[:], in_=idn16d.ap())
        t_idn32 = cst.tile([P, P], F32)
        nc.sync.dma_start(out=t_idn32[:], in_=idn32d.ap())
        t_obcol = cst.tile([P, 1], F32)
        nc.sync.dma_start(out=t_obcol[:], in_=obcold.ap())
        t_bcols = cst.tile([P, 2], F32)
        nc.sync.dma_start(out=t_bcols[:], in_=bcolsd.ap())
        # main-conv weights are needed only once the first gather lands; load
        # them after the other constants so they don't delay the offset conv.
        t_wre = cst.tile([P, 18, C], F16)
        nc.sync.dma_start(out=t_wre[:], in_=wre.ap().transpose([1, 0, 2]))

        # ---- stage A: offset conv -> off_sb [18, 2048] f32
        off_sb = sb.tile([P, NS], F32, tag="offsb")
        for blk in range(4):
            ps = psA.tile([P, 512], F32, tag="psoff")
            for t in range(18):
                kk, ch = t // 2, t % 2
                ky, kx = kk // 3, kk % 3
                rhs = t_xchw[:, ch, :].rearrange("p (r w) -> p r w", w=WP)[
                    :, blk * 8 + ky : blk * 8 + ky + 8, kx + 3 : kx + 3 + W
                ]
                nc.tensor.matmul(
                    ps[0:18, :],
                    lhsT=t_owre[:, t, :],
                    rhs=rhs,
                    start=(t == 0),
                    stop=(t == 17),
                )
            nc.scalar.add(off_sb[0:18, blk * 512 : (blk + 1) * 512], ps[0:18, :], t_obcol[0:18, :])
        if debug_outputs:
            nc.sync.dma_start(out=dbg_off.ap(), in_=off_sb[0:18, :])

        # ---- stage B: transpose to offT [128, 16, 18] f32
        offT = sb.tile([P, NT, 18], F32, tag="offT")
        for st in range(NT):
            pst = psA.tile([P, 18], F32, tag="pstr")
            nc.tensor.transpose(
                pst[:, 0:18],
                in_=off_sb[0:18, st * P : (st + 1) * P],
                identity=t_idn32[0:18, 0:18],
            )
            nc.vector.tensor_copy(offT[:, st, :], pst[:, 0:18])

        # ---- stage C: bilinear math (all [128, 16*18] f32)
        pP4 = sb.tile([P, NT, 18], F32, tag="pP4")
        nc.vector.tensor_add(pP4[:], offT[:], t_base[:].rearrange("p (s d) -> p s d", d=18))
        pc = sb.tile([P, NT, 18], F32, tag="pc")
        nc.vector.tensor_scalar(pc[:], pP4[:], 0.0, 70.999, op0=AL.max, op1=AL.min)
        # floor robust to the f32->i32 cast mode: cast(pc - 0.5) gives floor
        # under RNE (hw) but floor-1 for frac<0.5 under truncation (interp).
        # Correct with d = pc - cast; mask = (d >= 1); ipart += mask; frac = d
        # - mask. Exact-integer pc may keep frac==1.0 - equivalent bilinear.
        i32 = sb.tile([P, NT, 18], mybir.dt.int32, tag="i32")
        nc.vector.tensor_scalar_add(i32[:], pc[:], -0.5)
        ip0 = sb.tile([P, NT, 18], F32, tag="ip0")
        nc.vector.tensor_copy(ip0[:], i32[:])
        d0 = sb.tile([P, NT, 18], F32, tag="d0")
        nc.vector.tensor_sub(d0[:], pc[:], ip0[:])
        msk = sb.tile([P, NT, 18], F32, tag="msk")
        nc.vector.tensor_scalar(msk[:], d0[:], 1.0, None, op0=AL.is_ge)
        ipart = sb.tile([P, NT, 18], F32, tag="ipart")
        nc.vector.tensor_add(ipart[:], ip0[:], msk[:])
        frac = sb.tile([P, NT, 18], F32, tag="frac")
        nc.vector.tensor_sub(frac[:], d0[:], msk[:])
        omf = sb.tile([P, NT, 18], F32, tag="omf")
        nc.vector.tensor_scalar(omf[:], frac[:], -1.0, 1.0, op0=AL.mult, op1=AL.add)

        def ysl(t):  # [128, 16, 9] strided views (d = 2kk + {0:y, 1:x})
            return t[:].rearrange("p s (k two) -> p s k two", two=2)[:, :, :, 0]

        def xsl(t):
            return t[:].rearrange("p s (k two) -> p s k two", two=2)[:, :, :, 1]

        # w4 [128, 16, 9, 4]; corner order of the pair-table fetch:
        # q0=(y0,x0), q1=(y1,x0), q2=(y0,x1), q3=(y1,x1)
        w4 = sb.tile([P, NT, KK, 4], F32, tag="w4")
        nc.vector.tensor_mul(w4[:, :, :, 0], ysl(omf), xsl(omf))
        nc.vector.tensor_mul(w4[:, :, :, 1], ysl(frac), xsl(omf))
        nc.vector.tensor_mul(w4[:, :, :, 2], ysl(omf), xsl(frac))
        nc.vector.tensor_mul(w4[:, :, :, 3], ysl(frac), xsl(frac))
        if debug_outputs:
            nc.sync.dma_start(out=dbg_w4.ap(), in_=w4[:].rearrange("p a b c -> p (a b c)"))

        # idxf [128, 16, 9]: pair-table row = 72*(y0+4) + (x0+4)
        idxf = sb.tile([P, NT, KK], F32, tag="idxf")
        nc.vector.tensor_scalar_mul(idxf[:], ysl(ipart), 72.0)
        nc.vector.tensor_add(idxf[:], idxf[:], xsl(ipart))

        # ---- stage D: wrapped idx layout on-chip.
        # Gather call (h, kk) slot i = st8*128 + p needs its idx at wrapped
        # (r, s) = (i%16, i//16) = (p%16, st8*8 + p//16), replicated over the
        # 8 16-partition groups. Per half:
        #   T1: transpose idxf[:, h*8:(h+1)*8, :] [128, (st8 k)] -> U [72, 128]
        #   T2(a): transpose U cols [a*16 + (r%16)] x8 reps (stride-0 col AP)
        #          -> psum [128 (rep*16+r), 72 (st8 k)]
        #   copy (f32->i16): psum -> idxs16[h][:, kk, st8, a]
        idxs16 = []
        for h in range(2):
            psT1 = psT.tile([72, P], F32, tag="psT1", name=f"psT1_{h}")
            nc.tensor.transpose(
                psT1[:, :],
                in_=idxf[:, h * 8 : (h + 1) * 8, :].rearrange("p a b -> p (a b)"),
                identity=t_idn32[:, :],
            )
            U = sb.tile([72, P], F32, tag="U", name=f"U_{h}")
            nc.vector.tensor_copy(U[:], psT1[:])
            ih = sb.tile([P, KK, 8, 8], I16, tag="idxs16", name=f"idxs16_{h}")
            idxs16.append(ih)
            for a in range(8):
                psT2 = psT.tile([16, 72], F32, tag="psT2", name=f"psT2_{h}_{a}")
                nc.tensor.transpose(
                    psT2[:, :],
                    in_=U[:, a * 16 : (a + 1) * 16],
                    identity=t_idn32[0:72, 0:72],
                )
                nc.vector.tensor_copy(
                    ih[0:16, :, :, a].transpose([0, 2, 1]),
                    psT2[:].rearrange("p (s k) -> p s k", k=KK),
                )
            # replicate idx partitions 0:16 across 16:128 (dma_gather wants
            # the wrapped indices mirrored in every 16-partition group);
            # doubling tree: 16->32->64->128
            iap = ih[:]
            for n in (16, 32, 64):
                nc.sync.dma_start(
                    out=bass.AP(iap.tensor, iap.offset + n * 576, [[576, n], [1, 576]]),
                    in_=bass.AP(iap.tensor, iap.offset, [[576, n], [1, 576]]),
                )
        if debug_outputs:
            for h in range(2):
                nc.sync.dma_start(
                    out=dbg_idx.ap().rearrange("p (h n) -> p h n", h=2)[:, h, :],
                    in_=idxs16[h][:].rearrange("p a b c -> p (a b c)"),
                )

        psAB.close()  # free stage A/B/D PSUM banks
        psE = es.enter_context(tc.tile_pool(name="psE", bufs=2, space="PSUM"))
        psG = es.enter_context(tc.tile_pool(name="psG", bufs=1, space="PSUM"))

        # ---- stage E: gather + fused combine + transposes + streaming GEMM
        xpad_src = bass.AP(xpad2, 0, [[2 * C, TBL2 - 1], [1, 4 * C]])
        for h in range(2):
            sampled = smpp.tile([P, 18, 1024], F16, tag="sampled")
            pso = [
                [psG.tile([P, 512], F32, tag=f"pso{oh}{blk}", name=f"pso{oh}{blk}_{h}") for blk in range(2)]
                for oh in range(2)
            ]
            for kk in range(KK):
                gd = gpool.tile([P, 8, 1024], F16, tag="gd", name=f"gd_{h}_{kk}")
                nc.gpsimd.dma_gather(
                    gd[:],
                    xpad_src,
                    idxs16[h][:, kk, :, :],
                    num_idxs=1024,
                    num_idxs_reg=1024,
                    elem_size=4 * C,
                    elem_step=2 * C,
                )
                for g4 in range(2):
                    ptile = [
                        psE.tile([P, 512], F32, tag=f"pt{ch}", name=f"pt{ch}_{h}_{kk}_{g4}")
                        for ch in range(2)
                    ]
                    for i4 in range(4):
                        st8 = g4 * 4 + i4
                        st = h * 8 + st8
                        acc = sclp.tile([P, C], F16, tag="acc")
                        nc.vector.tensor_scalar_mul(
                            acc[:], gd[:, st8, 0:C], w4[:, st, kk, 0:1]
                        )
                        for q in range(1, 4):
                            nc.vector.scalar_tensor_tensor(
                                acc[:],
                                gd[:, st8, q * C : (q + 1) * C],
                                w4[:, st, kk, q : q + 1],
                                acc[:],
                                op0=AL.mult,
                                op1=AL.add,
                            )
                        for ch in range(2):
                            nc.tensor.matmul(
                                ptile[ch][:, i4 * P : (i4 + 1) * P],
                                lhsT=acc[:, ch * P : (ch + 1) * P],
                                rhs=t_idn16[:],
                                start=True,
                                stop=True,
                            )
                    for ch in range(2):
                        t = kk * 2 + ch
                        nc.scalar.copy(
                            sampled[:, t, g4 * 512 : (g4 + 1) * 512], ptile[ch][:]
                        )
                # streaming main GEMM: K-tiles t = 2kk, 2kk+1
                for ch in range(2):
                    t = kk * 2 + ch
                    for oh in range(2):
                        for blk in range(2):
                            nc.tensor.matmul(
                                pso[oh][blk][:],
                                lhsT=t_wre[:, t, oh * P : (oh + 1) * P],
                                rhs=sampled[:, t, blk * 512 : (blk + 1) * 512],
                                start=(t == 0),
                                stop=(t == 17),
                            )
            if debug_outputs:
                nc.sync.dma_start(
                    out=dbg_smp.ap().rearrange("p (t hh n) -> p t hh n", hh=2, n=1024)[:, :, h, :],
                    in_=sampled[:],
                )
            for oh in range(2):
                for blk in range(2):
                    ot = otp.tile([P, 512], F32, tag="ot")
                    nc.scalar.add(ot[:], pso[oh][blk][:], t_bcols[:, oh : oh + 1])
                    nc.sync.dma_start(
                        out=bass.AP(
                            out, oh * P * NS + h * 1024 + blk * 512, [[NS, P], [1, 512]]
                        ),
                        in_=ot[:],
                    )

    nc.compile()
    return nc


def host_prep(x, weight, bias, offset_w, offset_b):
    """Returns (in_maps list of 8 dicts, assemble fn)."""
    B = x.shape[0]
    xp = np.zeros((B, WP, WP, C), np.float16)
    xp[:, PAD : PAD + H, PAD : PAD + W, :] = x.transpose(0, 2, 3, 1)
    # pair table: row r = [pixel r | pixel r+72] so one 2KB fetch at rows
    # (r, r+1) yields all 4 bilinear corners.
    xpad2_b = []
    for b in range(B):
        flat = xp[b].reshape(NPIX, C)
        t2 = np.zeros((TBL2, 2 * C), np.float16)
        t2[: TBL2 - 1, 0:C] = flat[: TBL2 - 1]
        t2[: TBL2 - 1, C : 2 * C] = flat[72 : TBL2 - 1 + 72]
        xpad2_b.append(t2)
    # c-major padded image for the offset conv, per (b, hh): rows 32h+3 .. +37
    xcp = xp.transpose(0, 3, 1, 2).reshape(B, 2, P, WP, WP)  # [b, grp, 128, 72, 72]
    wre = np.ascontiguousarray(
        weight.reshape(C, 2, P, 3, 3).transpose(3, 4, 1, 2, 0).reshape(KK * 2, P, C)
    ).astype(np.float16)
    # t = kk*2 + ch ; value = weight[o, ch*128+i, ky, kx]
    owre = np.ascontiguousarray(
        offset_w.reshape(18, 2, P, 3, 3).transpose(3, 4, 1, 2, 0).reshape(KK * 2, P, 18)
    ).astype(np.float16)
    idn16 = np.eye(P, dtype=np.float16)
    idn32 = np.eye(P, dtype=np.float32)
    obcol = np.zeros((P, 1), np.float32)
    obcol[:18, 0] = offset_b
    bcols = np.asarray(bias, np.float32).reshape(2, P).T.copy()  # [128, 2]

    base_all = []
    for hh in range(2):
        base = np.zeros((P, NT, 18), np.float32)
        p = np.arange(P)
        for st in range(NT):
            n = st * P + p
            ho = 32 * hh + n // W
            wo = n % W
            for kk in range(KK):
                ky, kx = kk // 3, kk % 3
                base[:, st, 2 * kk + 0] = ky + ho - 1 + PAD
                base[:, st, 2 * kk + 1] = kx + wo - 1 + PAD
        base_all.append(base.reshape(P, NT * 18))

    in_maps = []
    for core in range(8):
        b, hh = core // 2, core % 2
        in_maps.append(
            {
                "xpad2": xpad2_b[b],
                "xchw": np.ascontiguousarray(
                    xcp[b, :, :, 32 * hh + 3 : 32 * hh + 37, :].reshape(2, P, 34 * WP)
                ),
                "wre": wre,
                "owre": owre,
                "basep4": base_all[hh],
                "idn16": idn16,
                "idn32": idn32,
                "obcol": obcol,
                "bcols": bcols,
            }
        )

    def assemble(results):
        y = np.empty((B, C, H, W), np.float32)
        for core in range(8):
            b, hh = core // 2, core % 2
            y[b, :, 32 * hh : 32 * (hh + 1), :] = results[core]["out"].reshape(C, HO, W)
        return y

    return in_maps, assemble


_CACHE = {}


def _maybe_reset_devices():
    # Clear any wedged accelerator state left by a previous crashed run.
    try:
        import ctypes
        import jax

        jax.devices()
        lib = ctypes.CDLL("/opt/axon/libaxon_pjrt.so")
        if hasattr(lib, "axon_reset"):
            lib.axon_reset.restype = ctypes.c_int64
            lib.axon_reset()
    except Exception:
        pass


def kernel(x, weight, bias, offset_w, offset_b, trace=False):
    if "nc" not in _CACHE:
        _maybe_reset_devices()
        _CACHE["nc"] = build()
    nc = _CACHE["nc"]
    in_maps, assemble = host_prep(
        np.asarray(x), np.asarray(weight), np.asarray(bias),
        np.asarray(offset_w), np.asarray(offset_b),
    )
    res = bass_utils.run_bass_kernel_spmd(
        nc, in_maps, core_ids=list(range(8)), trace=trace
    )
    out = assemble(res.results)
    _CACHE["last_exec_time_ns"] = res.exec_time_ns
    return out


# revision 11
# speedup vs baseline: 1.9624x; 1.4292x over previous
"""DeformConv2d TRN2 kernel: build + host prep + SPMD runner.

Layout/algorithm summary (per core; 8 cores = 4 batches x 2 row-halves):
  - setup runs per 1024-sample group h so the first gather starts early:
    offset conv (3x3, 18 out ch) as 18 K-tile matmuls over 2 row-blocks of
    the group -> offsets [18, 1024] -> PE-transpose to [n-part, 18] ->
    bilinear ints/fracs/weights on DVE ([128, 8, 18] per group).
  - gather indices are built ON-CHIP in the wrapped [16-partition, s] layout
    dma_gather requires: T1 PE-transpose idx [128, 72] -> [72, 128], 8 small
    T2 transposes [72, 16] -> [16, 72], strided DVE copies (f32->i16) into
    idxs16[16, kk, st8, a], then a 16->32->64->128 partition doubling tree of
    SBUF-SBUF DMAs replicates them. No DRAM roundtrip.
  - dma_gather on the PAIR table xpad2[r] = [pix r | pix r+72] with
    elem_size=1024, elem_step=512: ONE descriptor per (sample, tap) fetches
    all 4 bilinear corners (2KB). 9 gathers per group of num_idxs=1024.
  - combine: 4 independent DVE muls scale the corners by w4 (f16 2x mode),
    PE accumulates the 4 scaled corners via identity matmuls into PSUM
    (scaled transpose), Act copies PSUM->sampled [ch, n] f16.
  - main GEMM streams: K-tile t = kk*2+ch accumulated into 4 PSUM banks per
    group as each tap arrives; +bias, DMA out [256, 2048] f16.

Zero-padding of the table by 4 rows/cols emulates the reference's
valid-masking exactly for |excursion| <= 4; p is clamped to [0, 70.999] in
padded coords so larger offsets also read only zero-pad rows (-> exact 0).
"""

import sys

sys.path.insert(0, "/opt/trn_rl_repo")

import numpy as np

import bass_rust
import concourse.bass as bass
import concourse.bacc as bacc
import concourse.mybir as mybir
import concourse.tile as tile
from concourse import bass_utils

P = 128
KK = 9
C = 256
H = W = 64
HO = 32          # rows per core (half image)
NS = HO * W      # samples per core = 2048
NT = NS // P     # 16 subtiles of 128 samples
PAD = 4
WP = 72          # padded width/height
NPIX = WP * WP   # 5184 pixels
TBL2 = 5113      # pair-table rows (idx <= 5110, fetch spans rows idx..idx+1)
F16 = mybir.dt.float16
F32 = mybir.dt.float32
I16 = mybir.dt.int16


def build(debug_outputs=False):
    nc = bacc.Bacc("TRN2", num_devices=8, debug=False)

    xpad2 = nc.dram_tensor("xpad2", [TBL2, 2 * C], F16, kind="ExternalInput")
    xchw = nc.dram_tensor("xchw", [2, P, 34 * WP], F16, kind="ExternalInput")
    wre = nc.dram_tensor("wre", [18, P, C], F16, kind="ExternalInput")
    owre = nc.dram_tensor("owre", [P, 18 * 18], F16, kind="ExternalInput")
    basep4 = nc.dram_tensor("basep4", [P, NT * 18], F32, kind="ExternalInput")
    idn16d = nc.dram_tensor("idn16", [P, P], F16, kind="ExternalInput")
    idn32d = nc.dram_tensor("idn32", [P, P], F32, kind="ExternalInput")
    obcold = nc.dram_tensor("obcol", [P, 1], F32, kind="ExternalInput")
    bcolsd = nc.dram_tensor("bcols", [P, 2], F32, kind="ExternalInput")

    out = nc.dram_tensor("out", [C, NS], F16, kind="ExternalOutput")
    if debug_outputs:
        dbg_off = nc.dram_tensor("dbg_off", [18, NS], F32, kind="ExternalOutput")
        dbg_w4 = nc.dram_tensor("dbg_w4", [P, NT * KK * 4], F32, kind="ExternalOutput")
        dbg_idx = nc.dram_tensor("dbg_idx", [P, 2 * KK * 64], I16, kind="ExternalOutput")
        dbg_smp = nc.dram_tensor("dbg_smp", [P, 18 * NS], F16, kind="ExternalOutput")

    from contextlib import ExitStack

    AL = mybir.AluOpType

    with tile.TileContext(nc) as tc, ExitStack() as es:
        cst = es.enter_context(tc.tile_pool(name="cst", bufs=1))
        sb = es.enter_context(tc.tile_pool(name="sb", bufs=1))
        smpp = es.enter_context(tc.tile_pool(name="smp", bufs=2))
        gpool = es.enter_context(tc.tile_pool(name="gp", bufs=3))
        sclp = es.enter_context(tc.tile_pool(name="scl", bufs=4))
        otp = es.enter_context(tc.tile_pool(name="ot", bufs=4))
        psAB = ExitStack()
        psA = psAB.enter_context(tc.tile_pool(name="psA", bufs=2, space="PSUM"))
        psW1 = psAB.enter_context(tc.tile_pool(name="psW1", bufs=1, space="PSUM"))
        psT1p = psAB.enter_context(tc.tile_pool(name="psT1p", bufs=1, space="PSUM"))
        psT = psAB.enter_context(tc.tile_pool(name="psT", bufs=2, space="PSUM"))

        # ---- constants, ordered so the offset-conv dependencies land first
        t_idn16 = cst.tile([P, P], F16)
        nc.sync.dma_start(out=t_idn16[:], in_=idn16d.ap())
        t_xchw = cst.tile([P, 2, 34 * WP], F16)
        nc.sync.dma_start(out=t_xchw[:], in_=xchw.ap().transpose([1, 0, 2]))
        t_owre = cst.tile([P, 18, 18], F16)
        nc.sync.dma_start(out=t_owre[:], in_=owre.ap().rearrange("p (t d) -> p t d", d=18))
        t_obcol = cst.tile([P, 1], F32)
        nc.sync.dma_start(out=t_obcol[:], in_=obcold.ap())
        t_base = cst.tile([P, NT * 18], F32)
        nc.sync.dma_start(out=t_base[:], in_=basep4.ap())
        t_idn32 = cst.tile([P, P], F32)
        nc.sync.dma_start(out=t_idn32[:], in_=idn32d.ap())
        t_bcols = cst.tile([P, 2], F32)
        nc.sync.dma_start(out=t_bcols[:], in_=bcolsd.ap())

        # PE p-state warmup: a chain of tiny matmuls keeps PE busy from t~2us
        # so the offset conv runs at full clock (ramp needs ~3us of busy).
        psW = psW1.tile([P, P], F32, tag="psW")
        for i in range(40):
            nc.tensor.matmul(
                psW[0:64, 0:64], lhsT=t_idn16[:, 0:64], rhs=t_idn16[:, 0:64],
                start=(i == 0), stop=(i == 39),
            )

        # main-conv weights: needed only once the first gather lands
        t_wre = cst.tile([P, 18, C], F16)
        nc.sync.dma_start(out=t_wre[:], in_=wre.ap().transpose([1, 0, 2]))

        # ---- per-group setup: offset conv -> transpose -> bilinear -> idx
        off_sb = sb.tile([P, NS], F32, tag="offsb")
        offT = sb.tile([P, NT, 18], F32, tag="offT")
        pP4 = sb.tile([P, NT, 18], F32, tag="pP4")
        pc = sb.tile([P, NT, 18], F32, tag="pc")
        i32 = sb.tile([P, NT, 18], mybir.dt.int32, tag="i32")
        ip0 = sb.tile([P, NT, 18], F32, tag="ip0")
        d0 = sb.tile([P, NT, 18], F32, tag="d0")
        msk = sb.tile([P, NT, 18], F32, tag="msk")
        ipart = sb.tile([P, NT, 18], F32, tag="ipart")
        frac = sb.tile([P, NT, 18], F32, tag="frac")
        omf = sb.tile([P, NT, 18], F32, tag="omf")
        w4 = sb.tile([P, NT, KK, 4], F32, tag="w4")
        idxf = sb.tile([P, NT, KK], F32, tag="idxf")
        idxs16 = []

        def ysl(t, h):  # [128, 8, 9] strided views (d = 2kk + {0:y, 1:x})
            v = t[:].rearrange("p s (k two) -> p s k two", two=2)
            return v[:, 8 * h : 8 * (h + 1), :, 0]

        def xsl(t, h):
            v = t[:].rearrange("p s (k two) -> p s k two", two=2)
            return v[:, 8 * h : 8 * (h + 1), :, 1]

        for h in range(2):
            sl = slice(8 * h, 8 * (h + 1))
            # stage A: offset conv for this group's 2 row-blocks
            for b2 in range(2):
                blk = 2 * h + b2
                ps = psA.tile([P, 512], F32, tag="psoff")
                for t in range(18):
                    kk, ch = t // 2, t % 2
                    ky, kx = kk // 3, kk % 3
                    rhs = t_xchw[:, ch, :].rearrange("p (r w) -> p r w", w=WP)[
                        :, blk * 8 + ky : blk * 8 + ky + 8, kx + 3 : kx + 3 + W
                    ]
                    nc.tensor.matmul(
                        ps[0:18, :],
                        lhsT=t_owre[:, t, :],
                        rhs=rhs,
                        start=(t == 0),
                        stop=(t == 17),
                    )
                nc.scalar.add(
                    off_sb[0:18, blk * 512 : (blk + 1) * 512], ps[0:18, :], t_obcol[0:18, :]
                )
            # stage B: transpose to offT [128, st, 18]
            for st in range(8 * h, 8 * h + 8):
                pst = psA.tile([P, 18], F32, tag="pstr")
                nc.tensor.transpose(
                    pst[:, 0:18],
                    in_=off_sb[0:18, st * P : (st + 1) * P],
                    identity=t_idn32[0:18, 0:18],
                )
                nc.vector.tensor_copy(offT[:, st, :], pst[:, 0:18])
            # stage C: bilinear math on this group's slice [128, 8*18]
            bsl = t_base[:].rearrange("p (s d) -> p s d", d=18)[:, sl, :]
            nc.vector.tensor_add(pP4[:, sl, :], offT[:, sl, :], bsl)
            nc.vector.tensor_scalar(pc[:, sl, :], pP4[:, sl, :], 0.0, 70.999, op0=AL.max, op1=AL.min)
            # floor robust to the f32->i32 cast mode: cast(pc - 0.5) is floor
            # under RNE (hw) but floor-1 for frac<0.5 under truncation
            # (interp); fix with d0 = pc - cast, msk = (d0 >= 1).
            nc.vector.tensor_scalar_add(i32[:, sl, :], pc[:, sl, :], -0.5)
            nc.vector.tensor_copy(ip0[:, sl, :], i32[:, sl, :])
            nc.vector.tensor_sub(d0[:, sl, :], pc[:, sl, :], ip0[:, sl, :])
            nc.vector.tensor_scalar(msk[:, sl, :], d0[:, sl, :], 1.0, None, op0=AL.is_ge)
            nc.vector.tensor_add(ipart[:, sl, :], ip0[:, sl, :], msk[:, sl, :])
            nc.vector.tensor_sub(frac[:, sl, :], d0[:, sl, :], msk[:, sl, :])
            nc.vector.tensor_scalar(omf[:, sl, :], frac[:, sl, :], -1.0, 1.0, op0=AL.mult, op1=AL.add)
            # w4 corner order of the pair-table fetch:
            # q0=(y0,x0), q1=(y1,x0), q2=(y0,x1), q3=(y1,x1)
            nc.vector.tensor_mul(w4[:, sl, :, 0], ysl(omf, h), xsl(omf, h))
            nc.vector.tensor_mul(w4[:, sl, :, 1], ysl(frac, h), xsl(omf, h))
            nc.vector.tensor_mul(w4[:, sl, :, 2], ysl(omf, h), xsl(frac, h))
            nc.vector.tensor_mul(w4[:, sl, :, 3], ysl(frac, h), xsl(frac, h))
            # idxf [128, 8, 9]: pair-table row = 72*y0 + x0 (padded coords)
            nc.vector.tensor_scalar_mul(idxf[:, sl, :], ysl(ipart, h), 72.0)
            nc.vector.tensor_add(idxf[:, sl, :], idxf[:, sl, :], xsl(ipart, h))

            # stage D: wrapped idx layout on-chip. Gather call (h, kk) slot
            # i = st8*128 + p needs its idx at wrapped (r, s) = (i%16, i//16)
            # = (p%16, st8*8 + p//16), replicated over 16-partition groups.
            with tc.high_priority(offset=10000):
                psT1 = psT1p.tile([72, P], F32, tag="psT1", name=f"psT1_{h}")
                nc.tensor.transpose(
                    psT1[:, :],
                    in_=idxf[:, sl, :].rearrange("p a b -> p (a b)"),
                    identity=t_idn32[:, :],
                )
                U = sb.tile([72, P], F32, tag="U", name=f"U_{h}")
                nc.vector.tensor_copy(U[:], psT1[:])
                ih = sb.tile([P, KK, 8, 8], I16, tag="idxs16", name=f"idxs16_{h}")
                idxs16.append(ih)
                for a in range(8):
                    psT2 = psT.tile([16, 72], F32, tag="psT2", name=f"psT2_{h}_{a}")
                    nc.tensor.transpose(
                        psT2[:, :],
                        in_=U[:, a * 16 : (a + 1) * 16],
                        identity=t_idn32[0:72, 0:72],
                    )
                    nc.vector.tensor_copy(
                        ih[0:16, :, :, a].transpose([0, 2, 1]),
                        psT2[:].rearrange("p (s k) -> p s k", k=KK),
                    )
                # replicate idx partitions 0:16 across 16:128 (dma_gather
                # wants them mirrored in every 16-partition group)
                iap = ih[:]
                for n in (16, 32, 64):
                    nc.sync.dma_start(
                        out=bass.AP(iap.tensor, iap.offset + n * 576, [[576, n], [1, 576]]),
                        in_=bass.AP(iap.tensor, iap.offset, [[576, n], [1, 576]]),
                    )

        if debug_outputs:
            nc.sync.dma_start(out=dbg_off.ap(), in_=off_sb[0:18, :])
            nc.sync.dma_start(out=dbg_w4.ap(), in_=w4[:].rearrange("p a b c -> p (a b c)"))
            for h in range(2):
                nc.sync.dma_start(
                    out=dbg_idx.ap().rearrange("p (h n) -> p h n", h=2)[:, h, :],
                    in_=idxs16[h][:].rearrange("p a b c -> p (a b c)"),
                )

        psAB.close()  # free setup PSUM banks
        psE = es.enter_context(tc.tile_pool(name="psE", bufs=2, space="PSUM"))
        psG = es.enter_context(tc.tile_pool(name="psG", bufs=1, space="PSUM"))

        # ---- stage E: gather + scale + PSUM-accumulate transpose + GEMM
        xpad_src = bass.AP(xpad2, 0, [[2 * C, TBL2 - 1], [1, 4 * C]])
        for h in range(2):
            sampled = smpp.tile([P, 18, 1024], F16, tag="sampled")
            pso = [
                [psG.tile([P, 512], F32, tag=f"pso{oh}{blk}", name=f"pso{oh}{blk}_{h}") for blk in range(2)]
                for oh in range(2)
            ]
            for kk in range(KK):
                gd = gpool.tile([P, 8, 1024], F16, tag="gd", name=f"gd_{h}_{kk}")
                nc.gpsimd.dma_gather(
                    gd[:],
                    xpad_src,
                    idxs16[h][:, kk, :, :],
                    num_idxs=1024,
                    num_idxs_reg=1024,
                    elem_size=4 * C,
                    elem_step=2 * C,
                )
                for g4 in range(2):
                    ptile = [
                        psE.tile([P, 512], F32, tag=f"pt{ch}", name=f"pt{ch}_{h}_{kk}_{g4}")
                        for ch in range(2)
                    ]
                    for i4 in range(4):
                        st8 = g4 * 4 + i4
                        st = h * 8 + st8
                        scl4 = sclp.tile([P, 4, C], F16, tag="scl4")
                        for q in range(4):
                            nc.vector.tensor_scalar_mul(
                                scl4[:, q, :],
                                gd[:, st8, q * C : (q + 1) * C],
                                w4[:, st, kk, q : q + 1],
                            )
                        for ch in range(2):
                            for q in range(4):
                                nc.tensor.matmul(
                                    ptile[ch][:, i4 * P : (i4 + 1) * P],
                                    lhsT=scl4[:, q, ch * P : (ch + 1) * P],
                                    rhs=t_idn16[:],
                                    start=(q == 0),
                                    stop=(q == 3),
                                )
                    for ch in range(2):
                        t = kk * 2 + ch
                        nc.scalar.copy(
                            sampled[:, t, g4 * 512 : (g4 + 1) * 512], ptile[ch][:]
                        )
                # streaming main GEMM: K-tiles t = 2kk, 2kk+1
                for ch in range(2):
                    t = kk * 2 + ch
                    for oh in range(2):
                        for blk in range(2):
                            nc.tensor.matmul(
                                pso[oh][blk][:],
                                lhsT=t_wre[:, t, oh * P : (oh + 1) * P],
                                rhs=sampled[:, t, blk * 512 : (blk + 1) * 512],
                                start=(t == 0),
                                stop=(t == 17),
                            )
            if debug_outputs:
                nc.sync.dma_start(
                    out=dbg_smp.ap().rearrange("p (t hh n) -> p t hh n", hh=2, n=1024)[:, :, h, :],
                    in_=sampled[:],
                )
            for oh in range(2):
                for blk in range(2):
                    ot = otp.tile([P, 512], F16, tag="ot")
                    nc.scalar.add(ot[:], pso[oh][blk][:], t_bcols[:, oh : oh + 1])
                    nc.sync.dma_start(
                        out=bass.AP(
                            out, oh * P * NS + h * 1024 + blk * 512, [[NS, P], [1, 512]]
                        ),
                        in_=ot[:],
                    )

    nc.compile()
    return nc


def host_prep(x, weight, bias, offset_w, offset_b):
    """Returns (in_maps list of 8 dicts, assemble fn)."""
    B = x.shape[0]
    xp = np.zeros((B, WP, WP, C), np.float16)
    xp[:, PAD : PAD + H, PAD : PAD + W, :] = x.transpose(0, 2, 3, 1)
    # pair table: row r = [pixel r | pixel r+72] so one 2KB fetch at rows
    # (r, r+1) yields all 4 bilinear corners.
    xpad2_b = []
    for b in range(B):
        flat = xp[b].reshape(NPIX, C)
        t2 = np.zeros((TBL2, 2 * C), np.float16)
        t2[: TBL2 - 1, 0:C] = flat[: TBL2 - 1]
        t2[: TBL2 - 1, C : 2 * C] = flat[72 : TBL2 - 1 + 72]
        xpad2_b.append(t2)
    # c-major padded image for the offset conv, per (b, hh): rows 32h+3 .. +37
    xcp = xp.transpose(0, 3, 1, 2).reshape(B, 2, P, WP, WP)  # [b, grp, 128, 72, 72]
    wre = np.ascontiguousarray(
        weight.reshape(C, 2, P, 3, 3).transpose(3, 4, 1, 2, 0).reshape(KK * 2, P, C)
    ).astype(np.float16)
    # t = kk*2 + ch ; value = offset_w[o, ch*128+i, ky, kx]; packed [P, 18*18]
    owre = np.ascontiguousarray(
        offset_w.reshape(18, 2, P, 3, 3).transpose(2, 3, 4, 1, 0).reshape(P, 18 * 18)
    ).astype(np.float16)
    idn16 = np.eye(P, dtype=np.float16)
    idn32 = np.eye(P, dtype=np.float32)
    obcol = np.zeros((P, 1), np.float32)
    obcol[:18, 0] = offset_b
    bcols = np.asarray(bias, np.float32).reshape(2, P).T.copy()  # [128, 2]

    base_all = []
    for hh in range(2):
        base = np.zeros((P, NT, 18), np.float32)
        p = np.arange(P)
        for st in range(NT):
            n = st * P + p
            ho = 32 * hh + n // W
            wo = n % W
            for kk in range(KK):
                ky, kx = kk // 3, kk % 3
                base[:, st, 2 * kk + 0] = ky + ho - 1 + PAD
                base[:, st, 2 * kk + 1] = kx + wo - 1 + PAD
        base_all.append(base.reshape(P, NT * 18))

    in_maps = []
    for core in range(8):
        b, hh = core // 2, core % 2
        in_maps.append(
            {
                "xpad2": xpad2_b[b],
                "xchw": np.ascontiguousarray(
                    xcp[b, :, :, 32 * hh + 3 : 32 * hh + 37, :].reshape(2, P, 34 * WP)
                ),
                "wre": wre,
                "owre": owre,
                "basep4": base_all[hh],
                "idn16": idn16,
                "idn32": idn32,
                "obcol": obcol,
                "bcols": bcols,
            }
        )

    def assemble(results):
        y = np.empty((B, C, H, W), np.float32)
        for core in range(8):
            b, hh = core // 2, core % 2
            y[b, :, 32 * hh : 32 * (hh + 1), :] = (
                results[core]["out"].astype(np.float32).reshape(C, HO, W)
            )
        return y

    return in_maps, assemble


_CACHE = {}


def _maybe_reset_devices():
    # Clear any wedged accelerator state left by a previous crashed run.
    try:
        import ctypes
        import jax

        jax.devices()
        lib = ctypes.CDLL("/opt/axon/libaxon_pjrt.so")
        if hasattr(lib, "axon_reset"):
            lib.axon_reset.restype = ctypes.c_int64
            lib.axon_reset()
    except Exception:
        pass


def kernel(x, weight, bias, offset_w, offset_b, trace=False):
    if "nc" not in _CACHE:
        _maybe_reset_devices()
        _CACHE["nc"] = build()
    nc = _CACHE["nc"]
    in_maps, assemble = host_prep(
        np.asarray(x), np.asarray(weight), np.asarray(bias),
        np.asarray(offset_w), np.asarray(offset_b),
    )
    res = bass_utils.run_bass_kernel_spmd(
        nc, in_maps, core_ids=list(range(8)), trace=trace
    )
    out = assemble(res.results)
    _CACHE["last_exec_time_ns"] = res.exec_time_ns
    return out


# revision 19
# speedup vs baseline: 2.2030x; 1.1226x over previous
"""DeformConv2d TRN2 kernel: build + host prep + SPMD runner.

Layout/algorithm summary (per core; 8 cores = 4 batches x 2 row-halves):
  - setup runs per 1024-sample group h so the first gather starts early:
    offset conv (3x3, 18 out ch) as 18 K-tile matmuls over 2 row-blocks of
    the group -> offsets [18, 1024] -> PE-transpose to [n-part, 18] ->
    bilinear ints/fracs/weights on DVE ([128, 8, 18] per group).
  - gather indices are built ON-CHIP in the wrapped [16-partition, s] layout
    dma_gather requires: T1 PE-transpose idx [128, 72] -> [72, 128], 8 small
    T2 transposes [72, 16] -> [16, 72], strided DVE copies (f32->i16) into
    idxs16[16, kk, st8, a], then a 16->32->64->128 partition doubling tree of
    SBUF-SBUF DMAs replicates them. No DRAM roundtrip.
  - dma_gather on the PAIR table xpad2[r] = [pix r | pix r+72] with
    elem_size=1024, elem_step=512: ONE descriptor per (sample, tap) fetches
    all 4 bilinear corners (2KB). 9 gathers per group of num_idxs=1024.
  - combine: 4 independent DVE muls scale the corners by w4 (f16 2x mode),
    PE accumulates the 4 scaled corners via identity matmuls into PSUM
    (scaled transpose), Act copies PSUM->sampled [ch, n] f16.
  - main GEMM streams: K-tile t = kk*2+ch accumulated into 4 PSUM banks per
    group as each tap arrives; +bias, DMA out [256, 2048] f16.

Zero-padding of the table by 4 rows/cols emulates the reference's
valid-masking exactly for |excursion| <= 4; p is clamped to [0, 70.999] in
padded coords so larger offsets also read only zero-pad rows (-> exact 0).
"""

import sys

sys.path.insert(0, "/opt/trn_rl_repo")

import numpy as np

import bass_rust
import concourse.bass as bass
import concourse.bacc as bacc
import concourse.mybir as mybir
import concourse.tile as tile
from concourse import bass_utils

P = 128
KK = 9
C = 256
H = W = 64
HO = 32          # rows per core (half image)
NS = HO * W      # samples per core = 2048
NT = NS // P     # 16 subtiles of 128 samples
PAD = 4
WP = 72          # padded width/height
NPIX = WP * WP   # 5184 pixels
TBL2 = 5113      # pair-table rows (idx <= 5110, fetch spans rows idx..idx+1)
F16 = mybir.dt.float16
F32 = mybir.dt.float32
I16 = mybir.dt.int16


def build(debug_outputs=False):
    nc = bacc.Bacc("TRN2", num_devices=8, debug=False)

    xpad2 = nc.dram_tensor("xpad2", [TBL2, 2 * C], F16, kind="ExternalInput")
    xchw = nc.dram_tensor("xchw", [2, P, 34 * WP], F16, kind="ExternalInput")
    wre = nc.dram_tensor("wre", [18, P, C], F16, kind="ExternalInput")
    owre = nc.dram_tensor("owre", [P, 18 * 18], F16, kind="ExternalInput")
    basep4 = nc.dram_tensor("basep4", [P, NT * 18], F32, kind="ExternalInput")
    idn16d = nc.dram_tensor("idn16", [P, P], F16, kind="ExternalInput")
    idn32d = nc.dram_tensor("idn32", [P, P], F32, kind="ExternalInput")
    obcold = nc.dram_tensor("obcol", [P, 1], F32, kind="ExternalInput")
    bcolsd = nc.dram_tensor("bcols", [P, 2], F32, kind="ExternalInput")
    rseld = nc.dram_tensor("rsel", [P, 8 * P], F32, kind="ExternalInput")
    browd = nc.dram_tensor("brow", [1, 2 * P], F16, kind="ExternalInput")

    out = nc.dram_tensor("out", [C, NS], F16, kind="ExternalOutput")
    if debug_outputs:
        dbg_off = nc.dram_tensor("dbg_off", [18, NS], F32, kind="ExternalOutput")
        dbg_w4 = nc.dram_tensor("dbg_w4", [P, NT * KK * 4], F32, kind="ExternalOutput")
        dbg_idx = nc.dram_tensor("dbg_idx", [P, 2 * KK * 64], I16, kind="ExternalOutput")
        dbg_smp = nc.dram_tensor("dbg_smp", [P, 18 * NS], F16, kind="ExternalOutput")

    from contextlib import ExitStack

    AL = mybir.AluOpType

    with tile.TileContext(nc) as tc, ExitStack() as es:
        cst = es.enter_context(tc.tile_pool(name="cst", bufs=1))
        sb = es.enter_context(tc.tile_pool(name="sb", bufs=1))
        smpp = es.enter_context(tc.tile_pool(name="smp", bufs=2))
        gpool = es.enter_context(tc.tile_pool(name="gp", bufs=7))
        sclp = es.enter_context(tc.tile_pool(name="scl", bufs=4))
        otp = es.enter_context(tc.tile_pool(name="ot", bufs=4))
        psAB = ExitStack()
        psA = psAB.enter_context(tc.tile_pool(name="psA", bufs=2, space="PSUM"))
        psW1 = psAB.enter_context(tc.tile_pool(name="psW1", bufs=1, space="PSUM"))
        psT = psAB.enter_context(tc.tile_pool(name="psT", bufs=2, space="PSUM"))

        # ---- constants, ordered so the offset-conv dependencies land first
        t_idn16 = cst.tile([P, P], F16)
        nc.sync.dma_start(out=t_idn16[:], in_=idn16d.ap())
        t_xchw = cst.tile([P, 2, 34 * WP], F16)
        nc.sync.dma_start(
            out=t_xchw[:, :, : 18 * WP],
            in_=xchw.ap().transpose([1, 0, 2])[:, :, : 18 * WP],
        )
        t_owre = cst.tile([P, 18, 18], F16)
        nc.sync.dma_start(out=t_owre[:], in_=owre.ap().rearrange("p (t d) -> p t d", d=18))
        t_obcol = cst.tile([P, 1], F32)
        nc.sync.dma_start(out=t_obcol[:], in_=obcold.ap())
        t_base = cst.tile([P, NT * 18], F32)
        nc.sync.dma_start(out=t_base[:], in_=basep4.ap())
        t_idn32 = cst.tile([P, P], F32)
        nc.sync.dma_start(out=t_idn32[:], in_=idn32d.ap())
        t_bcols = cst.tile([P, 2], F32)
        nc.sync.dma_start(out=t_bcols[:], in_=bcolsd.ap())
        nc.sync.dma_start(
            out=t_xchw[:, :, 18 * WP :],
            in_=xchw.ap().transpose([1, 0, 2])[:, :, 18 * WP :],
        )
        t_rsel = cst.tile([P, 8, P], F32)
        nc.sync.dma_start(out=t_rsel[:], in_=rseld.ap().rearrange("p (a m) -> p a m", m=P))
        t_brow = cst.tile([1, 2, P], F16)
        nc.sync.dma_start(out=t_brow[:], in_=browd.ap().rearrange("o (h p) -> o h p", p=P))
        t_ones = cst.tile([1, 512], F16)
        nc.vector.memset(t_ones[:], 1.0)

        # PE p-state warmup: a chain of tiny matmuls keeps PE busy from t~2us
        # so the offset conv runs at full clock (ramp needs ~3us of busy).
        psW = psW1.tile([P, P], F32, tag="psW")
        for i in range(40):
            nc.tensor.matmul(
                psW[0:64, 0:64], lhsT=t_idn16[:, 0:64], rhs=t_idn16[:, 0:64],
                start=(i == 0), stop=(i == 39),
            )

        # main-conv weights: needed only once the first gather lands
        t_wre = cst.tile([P, 18, C], F16)
        nc.sync.dma_start(out=t_wre[:], in_=wre.ap().transpose([1, 0, 2]))

        # ---- per-group setup: offset conv -> transpose -> bilinear -> idx
        off_sb = sb.tile([P, NS], F32, tag="offsb")
        offT = sb.tile([P, NT, 18], F32, tag="offT")
        pP4 = sb.tile([P, NT, 18], F32, tag="pP4")
        pc = sb.tile([P, NT, 18], F32, tag="pc")
        i32 = sb.tile([P, NT, 18], mybir.dt.int32, tag="i32")
        ip0 = sb.tile([P, NT, 18], F32, tag="ip0")
        d0 = sb.tile([P, NT, 18], F32, tag="d0")
        msk = sb.tile([P, NT, 18], F32, tag="msk")
        ipart = sb.tile([P, NT, 18], F32, tag="ipart")
        frac = sb.tile([P, NT, 18], F32, tag="frac")
        omf = sb.tile([P, NT, 18], F32, tag="omf")
        w4 = sb.tile([P, NT, KK, 4], F32, tag="w4")
        idxf = sb.tile([P, NT, KK], F32, tag="idxf")
        idxs16 = []

        def ysl(t, h):  # [128, 8, 9] strided views (d = 2kk + {0:y, 1:x})
            v = t[:].rearrange("p s (k two) -> p s k two", two=2)
            return v[:, 8 * h : 8 * (h + 1), :, 0]

        def xsl(t, h):
            v = t[:].rearrange("p s (k two) -> p s k two", two=2)
            return v[:, 8 * h : 8 * (h + 1), :, 1]

        setup_ctx = ExitStack()
        setup_ctx.enter_context(tc.high_priority(offset=100000))
        for h in range(2):
            sl = slice(8 * h, 8 * (h + 1))
            # stage A: offset conv for this group's 2 row-blocks
            for b2 in range(2):
                blk = 2 * h + b2
                ps = psA.tile([P, 512], F32, tag="psoff")
                for t in range(18):
                    kk, ch = t // 2, t % 2
                    ky, kx = kk // 3, kk % 3
                    rhs = t_xchw[:, ch, :].rearrange("p (r w) -> p r w", w=WP)[
                        :, blk * 8 + ky : blk * 8 + ky + 8, kx + 3 : kx + 3 + W
                    ]
                    nc.tensor.matmul(
                        ps[0:18, :],
                        lhsT=t_owre[:, t, :],
                        rhs=rhs,
                        start=(t == 0),
                        stop=(t == 17),
                    )
                nc.scalar.add(
                    off_sb[0:18, blk * 512 : (blk + 1) * 512], ps[0:18, :], t_obcol[0:18, :]
                )
            # stage B: transpose to offT [128, st, 18]
            for st in range(8 * h, 8 * h + 8):
                pst = psA.tile([P, 18], F32, tag="pstr")
                nc.tensor.transpose(
                    pst[:, 0:18],
                    in_=off_sb[0:18, st * P : (st + 1) * P],
                    identity=t_idn32[0:18, 0:18],
                )
                nc.vector.tensor_copy(offT[:, st, :], pst[:, 0:18])
            # stage C: bilinear math on this group's slice [128, 8*18]
            bsl = t_base[:].rearrange("p (s d) -> p s d", d=18)[:, sl, :]
            nc.vector.tensor_add(pP4[:, sl, :], offT[:, sl, :], bsl)
            nc.vector.tensor_scalar(pc[:, sl, :], pP4[:, sl, :], 0.0, 70.999, op0=AL.max, op1=AL.min)
            # floor robust to the f32->i32 cast mode: cast(pc - 0.5) is floor
            # under RNE (hw) but floor-1 for frac<0.5 under truncation
            # (interp); fix with d0 = pc - cast, msk = (d0 >= 1).
            nc.vector.tensor_scalar_add(i32[:, sl, :], pc[:, sl, :], -0.5)
            nc.vector.tensor_copy(ip0[:, sl, :], i32[:, sl, :])
            nc.vector.tensor_sub(d0[:, sl, :], pc[:, sl, :], ip0[:, sl, :])
            nc.vector.tensor_scalar(msk[:, sl, :], d0[:, sl, :], 1.0, None, op0=AL.is_ge)
            nc.vector.tensor_add(ipart[:, sl, :], ip0[:, sl, :], msk[:, sl, :])
            nc.vector.tensor_sub(frac[:, sl, :], d0[:, sl, :], msk[:, sl, :])
            nc.vector.tensor_scalar(omf[:, sl, :], frac[:, sl, :], -1.0, 1.0, op0=AL.mult, op1=AL.add)
            # w4 corner order of the pair-table fetch:
            # q0=(y0,x0), q1=(y1,x0), q2=(y0,x1), q3=(y1,x1)
            nc.vector.tensor_mul(w4[:, sl, :, 0], ysl(omf, h), xsl(omf, h))
            nc.vector.tensor_mul(w4[:, sl, :, 1], ysl(frac, h), xsl(omf, h))
            nc.vector.tensor_mul(w4[:, sl, :, 2], ysl(omf, h), xsl(frac, h))
            nc.vector.tensor_mul(w4[:, sl, :, 3], ysl(frac, h), xsl(frac, h))
            # idxf [128, 8, 9]: pair-table row = 72*y0 + x0 (padded coords)
            nc.vector.tensor_scalar_mul(idxf[:, sl, :], ysl(ipart, h), 72.0)
            nc.vector.tensor_add(idxf[:, sl, :], idxf[:, sl, :], xsl(ipart, h))

            # stage D: wrapped idx layout on-chip. Gather call (h, kk) slot
            # i = st8*128 + p needs its idx at wrapped (r, s) = (i%16, i//16)
            # = (p%16, st8*8 + p//16), replicated over 16-partition groups.
            # One matmul per a with the constant selector R_a[p, m] = 1 iff
            # p == a*16 + m%16 yields psum[m, (st8 kk)] = idxf[a*16 + m%16,
            # (st8 kk)] - the wrapped layout, already replicated across all
            # 128 partitions.
            ih = sb.tile([P, KK, 8, 8], I16, tag="idxs16", name=f"idxs16_{h}")
            idxs16.append(ih)
            for a in range(8):
                psT2 = psT.tile([P, 72], F32, tag="psT2", name=f"psT2_{h}_{a}")
                nc.tensor.matmul(
                    psT2[:, :],
                    lhsT=t_rsel[:, a, :],
                    rhs=idxf[:, sl, :].rearrange("p a b -> p (a b)"),
                    start=True,
                    stop=True,
                )
                nc.vector.tensor_copy(
                    ih[:, :, :, a].transpose([0, 2, 1]),
                    psT2[:].rearrange("p (s k) -> p s k", k=KK),
                )

        setup_ctx.close()
        tc.cur_priority += 500000  # push stage E far behind setup in the ready heap
        if debug_outputs:
            nc.sync.dma_start(out=dbg_off.ap(), in_=off_sb[0:18, :])
            nc.sync.dma_start(out=dbg_w4.ap(), in_=w4[:].rearrange("p a b c -> p (a b c)"))
            for h in range(2):
                nc.sync.dma_start(
                    out=dbg_idx.ap().rearrange("p (h n) -> p h n", h=2)[:, h, :],
                    in_=idxs16[h][:].rearrange("p a b c -> p (a b c)"),
                )

        psAB.close()  # free setup PSUM banks
        psE = es.enter_context(tc.tile_pool(name="psE", bufs=2, space="PSUM"))
        psG = es.enter_context(tc.tile_pool(name="psG", bufs=1, space="PSUM"))

        # ---- stage E: gather + scale + PSUM-accumulate transpose + GEMM
        xpad_src = bass.AP(xpad2, 0, [[2 * C, TBL2 - 1], [1, 4 * C]])
        for h in range(2):
            sampled = smpp.tile([P, 18, 1024], F16, tag="sampled")
            pso = [
                [psG.tile([P, 512], F32, tag=f"pso{oh}{blk}", name=f"pso{oh}{blk}_{h}") for blk in range(2)]
                for oh in range(2)
            ]
            # bias via rank-1 start matmul so the final PSUM read needs no add
            for oh in range(2):
                for blk in range(2):
                    nc.tensor.matmul(
                        pso[oh][blk][:],
                        lhsT=t_brow[0:1, oh, :],
                        rhs=t_ones[0:1, :],
                        start=True,
                        stop=False,
                    )
            pend = []
            for kk in range(KK):
                for g4 in range(2):
                    gd = gpool.tile([P, 4, 1024], F16, tag="gd", name=f"gd_{h}_{kk}_{g4}")
                    nc.gpsimd.dma_gather(
                        gd[:],
                        xpad_src,
                        idxs16[h][:, kk, g4 * 4 : (g4 + 1) * 4, :],
                        num_idxs=512,
                        num_idxs_reg=512,
                        elem_size=4 * C,
                        elem_step=2 * C,
                    )
                    ptile = [
                        psE.tile([P, 512], F32, tag=f"pt{ch}", name=f"pt{ch}_{h}_{kk}_{g4}")
                        for ch in range(2)
                    ]
                    for i4 in range(4):
                        st8 = g4 * 4 + i4
                        st = h * 8 + st8
                        scl4 = sclp.tile([P, 4, C], F16, tag="scl4")
                        for q in range(4):
                            nc.vector.tensor_scalar_mul(
                                scl4[:, q, :],
                                gd[:, i4, q * C : (q + 1) * C],
                                w4[:, st, kk, q : q + 1],
                            )
                        for ch in range(2):
                            for q in range(4):
                                nc.tensor.matmul(
                                    ptile[ch][:, i4 * P : (i4 + 1) * P],
                                    lhsT=scl4[:, q, ch * P : (ch + 1) * P],
                                    rhs=t_idn16[:],
                                    start=(q == 0),
                                    stop=(q == 3),
                                )
                    for ch in range(2):
                        t = kk * 2 + ch
                        nc.scalar.copy(
                            sampled[:, t, g4 * 512 : (g4 + 1) * 512], ptile[ch][:]
                        )
                    # streaming main GEMM, delayed 2 blocks so PE never
                    # stalls inline on the Act sampled-copy
                    pend.append((kk, g4))
                    if len(pend) > 2:
                        dk, dg = pend.pop(0)
                        for ch in range(2):
                            t = dk * 2 + ch
                            for oh in range(2):
                                nc.tensor.matmul(
                                    pso[oh][dg][:],
                                    lhsT=t_wre[:, t, oh * P : (oh + 1) * P],
                                    rhs=sampled[:, t, dg * 512 : (dg + 1) * 512],
                                    start=False,
                                    stop=(t == 17),
                                )
            for dk, dg in pend:
                for ch in range(2):
                    t = dk * 2 + ch
                    for oh in range(2):
                        nc.tensor.matmul(
                            pso[oh][dg][:],
                            lhsT=t_wre[:, t, oh * P : (oh + 1) * P],
                            rhs=sampled[:, t, dg * 512 : (dg + 1) * 512],
                            start=False,
                            stop=(t == 17),
                        )
            if debug_outputs:
                nc.sync.dma_start(
                    out=dbg_smp.ap().rearrange("p (t hh n) -> p t hh n", hh=2, n=1024)[:, :, h, :],
                    in_=sampled[:],
                )
            for oh in range(2):
                for blk in range(2):
                    ot = otp.tile([P, 512], F16, tag="ot", name=f"ot_{h}_{oh}_{blk}")
                    if oh == 0:
                        nc.scalar.copy(ot[:], pso[oh][blk][:])
                    else:
                        nc.vector.tensor_copy(ot[:], pso[oh][blk][:])
                    nc.sync.dma_start(
                        out=bass.AP(
                            out, oh * P * NS + h * 1024 + blk * 512, [[NS, P], [1, 512]]
                        ),
                        in_=ot[:],
                    )

    nc.compile()
    return nc


def host_prep(x, weight, bias, offset_w, offset_b):
    """Returns (in_maps list of 8 dicts, assemble fn)."""
    B = x.shape[0]
    xp = np.zeros((B, WP, WP, C), np.float16)
    xp[:, PAD : PAD + H, PAD : PAD + W, :] = x.transpose(0, 2, 3, 1)
    # pair table: row r = [pixel r | pixel r+72] so one 2KB fetch at rows
    # (r, r+1) yields all 4 bilinear corners.
    xpad2_b = []
    for b in range(B):
        flat = xp[b].reshape(NPIX, C)
        t2 = np.zeros((TBL2, 2 * C), np.float16)
        t2[: TBL2 - 1, 0:C] = flat[: TBL2 - 1]
        t2[: TBL2 - 1, C : 2 * C] = flat[72 : TBL2 - 1 + 72]
        xpad2_b.append(t2)
    # c-major padded image for the offset conv, per (b, hh): rows 32h+3 .. +37
    xcp = xp.transpose(0, 3, 1, 2).reshape(B, 2, P, WP, WP)  # [b, grp, 128, 72, 72]
    wre = np.ascontiguousarray(
        weight.reshape(C, 2, P, 3, 3).transpose(3, 4, 1, 2, 0).reshape(KK * 2, P, C)
    ).astype(np.float16)
    # t = kk*2 + ch ; value = offset_w[o, ch*128+i, ky, kx]; packed [P, 18*18]
    owre = np.ascontiguousarray(
        offset_w.reshape(18, 2, P, 3, 3).transpose(2, 3, 4, 1, 0).reshape(P, 18 * 18)
    ).astype(np.float16)
    idn16 = np.eye(P, dtype=np.float16)
    idn32 = np.eye(P, dtype=np.float32)
    obcol = np.zeros((P, 1), np.float32)
    obcol[:18, 0] = offset_b
    # selector for the wrapped-idx matmuls: rsel[p, a, m] = 1 iff p == a*16 + m%16
    rsel = np.zeros((P, 8, P), np.float32)
    pp = np.arange(P)
    for a in range(8):
        for m in range(P):
            rsel[a * 16 + m % 16, a, m] = 1.0
    rsel = rsel.reshape(P, 8 * P)
    brow = np.asarray(bias, np.float16).reshape(1, 2 * P)
    bcols = np.asarray(bias, np.float32).reshape(2, P).T.copy()  # [128, 2]

    base_all = []
    for hh in range(2):
        base = np.zeros((P, NT, 18), np.float32)
        p = np.arange(P)
        for st in range(NT):
            n = st * P + p
            ho = 32 * hh + n // W
            wo = n % W
            for kk in range(KK):
                ky, kx = kk // 3, kk % 3
                base[:, st, 2 * kk + 0] = ky + ho - 1 + PAD
                base[:, st, 2 * kk + 1] = kx + wo - 1 + PAD
        base_all.append(base.reshape(P, NT * 18))

    in_maps = []
    for core in range(8):
        b, hh = core // 2, core % 2
        in_maps.append(
            {
                "xpad2": xpad2_b[b],
                "xchw": np.ascontiguousarray(
                    xcp[b, :, :, 32 * hh + 3 : 32 * hh + 37, :].reshape(2, P, 34 * WP)
                ),
                "wre": wre,
                "owre": owre,
                "basep4": base_all[hh],
                "idn16": idn16,
                "idn32": idn32,
                "obcol": obcol,
                "bcols": bcols,
                "rsel": rsel,
                "brow": brow,
            }
        )

    def assemble(results):
        y = np.empty((B, C, H, W), np.float32)
        for core in range(8):
            b, hh = core // 2, core % 2
            y[b, :, 32 * hh : 32 * (hh + 1), :] = (
                results[core]["out"].astype(np.float32).reshape(C, HO, W)
            )
        return y

    return in_maps, assemble


_CACHE = {}


def _maybe_reset_devices():
    # Clear any wedged accelerator state left by a previous crashed run.
    try:
        import ctypes
        import jax

        jax.devices()
        lib = ctypes.CDLL("/opt/axon/libaxon_pjrt.so")
        if hasattr(lib, "axon_reset"):
            lib.axon_reset.restype = ctypes.c_int64
            lib.axon_reset()
    except Exception:
        pass


def kernel(x, weight, bias, offset_w, offset_b, trace=False):
    if "nc" not in _CACHE:
        _maybe_reset_devices()
        _CACHE["nc"] = build()
    nc = _CACHE["nc"]
    in_maps, assemble = host_prep(
        np.asarray(x), np.asarray(weight), np.asarray(bias),
        np.asarray(offset_w), np.asarray(offset_b),
    )
    res = bass_utils.run_bass_kernel_spmd(
        nc, in_maps, core_ids=list(range(8)), trace=trace
    )
    out = assemble(res.results)
    _CACHE["last_exec_time_ns"] = res.exec_time_ns
    return out
